# revision 5
# baseline (speedup 1.0000x reference)
"""RBF causal attention (unnormalized, no softmax denominator) on 8 Trainium2 NeuronCores.

Problem: B=2 H=16 N=2048 D=128 fp32.
  P[m,n] = exp(-s*||q_m - k_n||^2) for m >= n else 0;  O = P @ V
         = exp(2s*(q.k) - s*|k|^2) * exp(-s*|q|^2) masked causally.

Sharding: (b*h) = 32 independent slices -> 4 per core across 8 cores, no comms.

v3: host-permuted SBUF-native layouts + piece-granular input tiles.
  - host precomputes ksqb = -s*|k|^2 and eq = exp(-s*|q|^2) in fp32 from the
    ORIGINAL fp32 q/k (exact, matches reference); q/k/v ship as bf16.
  - all device I/O is pre-permuted host-side to the SBUF tile layout
    [128 partitions, blocks, D], so every DMA descriptor moves contiguous
    4KB-per-partition runs (vs 256B runs for the (n d) layout) - ~2.5x
    effective HBM bandwidth; the output is un-permuted host-side.
  - q/k/v load into FOUR separate 4-block piece tiles each, so a transpose
    (whole-tile dependency tracking) only waits for the 131KB piece it
    actually reads - cuts the cold-start ramp.
  - DMA issue cost (~650ns/issue on the issuing sequencer) is spread:
    q+stats+output on sync, k on vector, v on gpsimd (scalar helps cold).

Single flat software-pipelined stream over (slice, strip, bn):
  - PE-transpose Q,K 128x128 blocks (bf16) staged in PSUM, DVE-drained to
    SBUF (bf16 = 2x DVE rate); transpose groups interleave into the ACT-bound
    bn stream as PE filler
  - two m-strips of 1024 per slice; per (strip, bn):
      ST[n, m] = KT_bn.T @ QT strip   (bf16 matmul, fp32 PSUM, 512 halves)
      PT = exp(2s*ST - s*k_sq[n])     (ACT, bf16 out, per-partition bias)
      diag block: PT *= upper-tri mask (DVE/GPSIMD, bf16)
      per m-block j >= bn: ACC[m, d] += PT_block.T @ vb_bn  (bf16 matmul,
        P^T block as stationary -> output lands directly in [m, d] layout)
      PSUM allows one open accumulation group per 2KB bank: ACC is two
      1-bank tiles (4 m-blocks each); open at the bank's first write,
      close at its last diag, then DVE-drain the quad with eq scale
  - output DMA per strip half
"""

import os
import sys

import numpy as np

_TRN_REPO = "/opt/trn_rl_repo"
if os.path.isdir(_TRN_REPO) and _TRN_REPO not in sys.path:
    sys.path.insert(0, _TRN_REPO)

import concourse.bass as bass  # noqa: E402
import concourse.mybir as mybir  # noqa: E402
import concourse.tile as tile  # noqa: E402
from concourse import bacc  # noqa: E402
from concourse.bass_utils import run_bass_kernel_spmd  # noqa: E402
from concourse.masks import make_identity, make_upper_triangular  # noqa: E402

B, H, N, D = 2, 16, 2048, 128
SM_SCALE = 0.08838834764831845  # 1/sqrt(D)
NCORES = 8
SLICES = (B * H) // NCORES  # per core
NT = N // 128  # 16 row-blocks per slice

F32 = mybir.dt.float32
BF16 = mybir.dt.bfloat16

_nc_cache = None


def _build_nc():
    nc = bacc.Bacc("TRN2", target_bir_lowering=False, debug=False, num_devices=NCORES)

    # all I/O pre-permuted host-side: [p, t, d] with n = 128*t + p
    q_dram = nc.dram_tensor("q", [SLICES, 128, NT * D], BF16, kind="ExternalInput").ap()
    k_dram = nc.dram_tensor("k", [SLICES, 128, NT * D], BF16, kind="ExternalInput").ap()
    v_dram = nc.dram_tensor("v", [SLICES, 128, NT * D], BF16, kind="ExternalInput").ap()
    # exact fp32 host-side stats: ksqb = -s*|k|^2, eq = exp(-s*|q|^2)
    ksqb_dram = nc.dram_tensor("ksqb", [SLICES, 128, NT], F32, kind="ExternalInput").ap()
    eq_dram = nc.dram_tensor("eq", [SLICES, 128, NT], F32, kind="ExternalInput").ap()
    o_dram = nc.dram_tensor("o", [SLICES, 128, NT * D], BF16, kind="ExternalOutput").ap()

    with tile.TileContext(nc) as tc:
        singles = tc.alloc_tile_pool(name="singles", bufs=1)
        io = tc.alloc_tile_pool(name="io", bufs=2)
        tqk = tc.alloc_tile_pool(name="tqk", bufs=2)
        smalls = tc.alloc_tile_pool(name="smalls", bufs=2)
        ptp = tc.alloc_tile_pool(name="ptp", bufs=6)
        outp = tc.alloc_tile_pool(name="outp", bufs=2)
        # 8-bank PSUM budget: stp 3 x 2 banks (QK tiles + transpose stages
        # share the ring), accp 2 x 1 bank
        stp = tc.alloc_tile_pool(name="stp", bufs=3, space="PSUM")
        accp = tc.alloc_tile_pool(name="accp", bufs=2, space="PSUM")

        ident = singles.tile([128, 128], F32)
        make_identity(nc, ident)
        identb = singles.tile([128, 128], BF16)
        nc.vector.tensor_copy(identb, ident)
        # tri[n, m] = 1.0 where m >= n else 0.0 (keep causal, [n, m] layout)
        tri_f = singles.tile([128, 128], F32)
        make_upper_triangular(nc, tri_f, val=1.0, diag=True)
        tri_bf = singles.tile([128, 128], BF16)
        nc.vector.tensor_copy(tri_bf, tri_f)

        def dma_piece(s, dram, which, g, eng):
            """One 4-block piece: [128, 512] contiguous in dram and SBUF."""
            t = io.tile([128, 4, 128], BF16, name=f"{which}{g}_{s}", tag=f"{which}{g}")
            eng.dma_start(out=t, in_=dram[s][:, 512 * g : 512 * (g + 1)])
            in_tiles[s][which].append(t)

        def dma_stat(s, dram, tag, eng):
            t = smalls.tile([128, NT], F32, name=f"{tag}{s}", tag=tag)
            eng.dma_start(out=t, in_=dram[s])
            stats[s][tag] = t

        def emit_in_dma(s):
            """First-half input DMAs (pieces 0,1 of q/k/v + stats)."""
            in_tiles[s] = {"k": [], "q": [], "v": []}
            stats[s] = {}
            kq, vq = (nc.scalar, nc.gpsimd) if s == 0 else (nc.gpsimd, nc.gpsimd)
            dma_stat(s, ksqb_dram, "ksqb", kq)
            dma_piece(s, k_dram, "k", 0, kq)
            dma_piece(s, q_dram, "q", 0, nc.sync)
            dma_piece(s, k_dram, "k", 1, kq)
            dma_piece(s, q_dram, "q", 1, nc.sync)
            dma_stat(s, eq_dram, "eq", nc.sync)
            dma_piece(s, v_dram, "v", 0, vq)
            dma_piece(s, v_dram, "v", 1, vq)

        def emit_in_dma2(s):
            """Second-half input DMAs (pieces 2,3)."""
            kq = nc.scalar if s == 0 else nc.gpsimd
            dma_piece(s, k_dram, "k", 2, kq)
            dma_piece(s, q_dram, "q", 2, nc.sync)
            dma_piece(s, k_dram, "k", 3, kq)
            dma_piece(s, q_dram, "q", 3, nc.sync)
            dma_piece(s, v_dram, "v", 2, nc.gpsimd)
            dma_piece(s, v_dram, "v", 3, nc.gpsimd)

        def transpose_group(s, which, g):
            """PE-transpose piece g of k/q into kt/qt[:, 512g:...] via a PSUM
            stage (shared stp ring), drained by DVE (bf16 2x)."""
            src = sl[s]["io"][which][g]
            dst = sl[s]["kt"] if which == "k" else sl[s]["qt"]
            stg = stp.tile([128, 512], BF16, name=f"tsg{s}_{which}_{g}", tag="st")
            for j in range(4):
                nc.tensor.transpose(
                    stg[:, 128 * j : 128 * (j + 1)], src[:, j, :], identb
                )
            nc.vector.tensor_copy(dst[:, 512 * g : 512 * (g + 1)], stg)

        sl = {}
        stats = {}
        in_tiles = {}

        def alloc_slice(s):
            st = stats.pop(s)
            sl[s] = dict(
                io=in_tiles.pop(s),
                kt=tqk.tile([128, N], BF16, name=f"kt{s}", tag="kt"),
                qt=tqk.tile([128, N], BF16, name=f"qt{s}", tag="qt"),
                ksqb=st["ksqb"],
                eq=st["eq"],
                o_out=outp.tile([128, NT, 128], BF16, name=f"oo{s}", tag="oo"),
            )

        def qk_exp(s, p, bn):
            """ST = KT_bn.T @ QT strip; PT = bf16 exp(2s*ST - s*k_sq); mask."""
            kt, qt = sl[s]["kt"], sl[s]["qt"]
            off = max(0, 128 * bn - 1024 * p)
            stt = stp.tile([128, 1024], F32, name=f"st{s}_{p}_{bn}", tag="st")
            a = off
            while a < 1024:
                b = 512 if a < 512 else 1024
                nc.tensor.matmul(
                    stt[:, a:b],
                    kt[:, 128 * bn : 128 * (bn + 1)],
                    qt[:, 1024 * p + a : 1024 * p + b],
                    start=True,
                    stop=True,
                )
                a = b
            ptt = ptp.tile([128, 1024], BF16, name=f"pt{s}_{p}_{bn}", tag="pt")
            nc.scalar.activation(
                ptt[:, off:], stt[:, off:], mybir.ActivationFunctionType.Exp,
                bias=sl[s]["ksqb"][:, bn : bn + 1], scale=2.0 * SM_SCALE,
            )
            if bn >= 8 * p:
                eng = nc.vector if p == 0 else nc.gpsimd
                eng.tensor_mul(
                    ptt[:, off : off + 128], ptt[:, off : off + 128], tri_bf
                )
            return ptt

        # flat (s, p, bn) pair stream
        pairs = [
            (s, p, bn) for s in range(SLICES) for p in (0, 1)
            for bn in range(8 * p + 8)
        ]
        pidx = {t: i for i, t in enumerate(pairs)}

        fillers = {}

        def add_filler(key, fn):
            fillers.setdefault(pidx[key], []).append(fn)

        for s in range(SLICES):
            # second-half transposes run in this slice's strip0 (qt second
            # half needed at (s,1,0); kt blocks 8+ needed at (s,1,8))
            add_filler((s, 0, 2), lambda s=s: transpose_group(s, "k", 2))
            add_filler((s, 0, 3), lambda s=s: transpose_group(s, "k", 3))
            add_filler((s, 0, 4), lambda s=s: transpose_group(s, "q", 2))
            add_filler((s, 0, 5), lambda s=s: transpose_group(s, "q", 3))
            if s + 1 < SLICES:
                add_filler((s, 0, 0), lambda s=s: emit_in_dma(s + 1))
                add_filler((s, 0, 1), lambda s=s: emit_in_dma2(s + 1))
                add_filler((s, 1, 0), lambda s=s: alloc_slice(s + 1))
                add_filler((s, 1, 2), lambda s=s: transpose_group(s + 1, "q", 0))
                add_filler((s, 1, 4), lambda s=s: transpose_group(s + 1, "k", 0))
                add_filler((s, 1, 6), lambda s=s: transpose_group(s + 1, "q", 1))
                add_filler((s, 1, 8), lambda s=s: transpose_group(s + 1, "k", 1))

        # cold start: slice 0 DMA, first-half transposes
        emit_in_dma(0)
        emit_in_dma2(0)
        alloc_slice(0)
        transpose_group(0, "q", 0)
        transpose_group(0, "k", 0)
        transpose_group(0, "q", 1)
        transpose_group(0, "k", 1)

        def emit_out(s, p):
            nc.sync.dma_start(
                out=o_dram[s][:, 1024 * p : 1024 * (p + 1)],
                in_=sl[s]["o_out"][:, 8 * p : 8 * (p + 1)],
            )

        outq = []
        acc = {}
        pts = {0: qk_exp(*pairs[0]), 1: qk_exp(*pairs[1])}
        for i, (s, p, bn) in enumerate(pairs):
            if i + 2 < len(pairs):
                pts[i + 2] = qk_exp(*pairs[i + 2])
            ptt = pts.pop(i)
            if bn == 0:
                acc[0] = accp.tile([128, 4, 128], F32, name=f"acA{s}_{p}", tag="acc")
                acc[1] = accp.tile([128, 4, 128], F32, name=f"acB{s}_{p}", tag="acc")
            j0 = max(0, bn - 8 * p)
            js = list(range(j0, 8))
            if bn >= max(8 * p, 1) and len(js) > 1:
                # diag block last so its mask is off the PE critical path
                js = js[1:] + js[:1]
            vt = sl[s]["io"]["v"][bn // 4]
            for j in js:
                bm = 8 * p + j
                bank_last = (j % 4 == 3) and bn == bm
                nc.tensor.matmul(
                    acc[j // 4][:, j % 4, :],
                    ptt[:, 128 * j : 128 * (j + 1)],
                    vt[:, bn % 4, :],
                    start=(bn == 0 and j % 4 == 0),
                    stop=bank_last,
                )
                if bank_last:
                    for jj in range(j - 3, j + 1):
                        nc.vector.tensor_scalar_mul(
                            sl[s]["o_out"][:, 8 * p + jj, :],
                            acc[j // 4][:, jj % 4, :],
                            sl[s]["eq"][:, 8 * p + jj : 8 * p + jj + 1],
                        )
            for fn in fillers.get(i, ()):
                fn()
            if outq and outq[0][0] + 2 <= i:
                _, os_, op_ = outq.pop(0)
                emit_out(os_, op_)
            if bn == 8 * p + 7:  # strip end -> queue output DMA for this half
                outq.append((i, s, p))

        for _, os_, op_ in outq:
            # tail strips: split per bank-quad so the first half's DMA
            # overlaps the final PVs/drains instead of waiting for them
            for quad in (2 * op_, 2 * op_ + 1):
                nc.sync.dma_start(
                    out=o_dram[os_][:, 512 * quad : 512 * (quad + 1)],
                    in_=sl[os_]["o_out"][:, 4 * quad : 4 * (quad + 1)],
                )

        for pool in (accp, stp, outp, ptp, smalls, tqk, io, singles):
            pool.release()

    nc.compile()
    return nc


def _get_nc():
    global _nc_cache
    if _nc_cache is None:
        _nc_cache = _build_nc()
    return _nc_cache


def run(q, k, v, trace=False):
    q = np.ascontiguousarray(np.asarray(q, dtype=np.float32))
    k = np.ascontiguousarray(np.asarray(k, dtype=np.float32))
    v = np.ascontiguousarray(np.asarray(v, dtype=np.float32))
    import ml_dtypes

    S = B * H
    qf = q.reshape(S, N, D)
    kf = k.reshape(S, N, D)
    # exact fp32 statistics from the original fp32 data, permuted [p, t]
    ksqb = (-SM_SCALE * np.einsum("snd,snd->sn", kf, kf)).astype(np.float32)
    eq = np.exp(-SM_SCALE * np.einsum("snd,snd->sn", qf, qf)).astype(np.float32)
    ksqb = ksqb.reshape(S, NT, 128).transpose(0, 2, 1)
    eq = eq.reshape(S, NT, 128).transpose(0, 2, 1)

    def perm(x):  # [S, N, D] -> [S, 128, NT*D] bf16, n = 128*t + p
        return np.ascontiguousarray(
            x.reshape(S, NT, 128, D).transpose(0, 2, 1, 3).reshape(S, 128, NT * D)
        ).astype(ml_dtypes.bfloat16)

    qb, kb, vb = perm(qf), perm(kf), perm(v.reshape(S, N, D))
    nc = _get_nc()
    in_maps = [
        {
            "q": qb[SLICES * i : SLICES * (i + 1)],
            "k": kb[SLICES * i : SLICES * (i + 1)],
            "v": vb[SLICES * i : SLICES * (i + 1)],
            "ksqb": np.ascontiguousarray(ksqb[SLICES * i : SLICES * (i + 1)]),
            "eq": np.ascontiguousarray(eq[SLICES * i : SLICES * (i + 1)]),
        }
        for i in range(NCORES)
    ]
    res = run_bass_kernel_spmd(nc, in_maps, core_ids=list(range(NCORES)), trace=trace)
    out = np.concatenate(
        [np.asarray(res.results[i]["o"]) for i in range(NCORES)], axis=0
    )
    # un-permute [S, 128, NT*D] -> [S, N, D]
    out = (
        out.reshape(S, 128, NT, D).transpose(0, 2, 1, 3).reshape(S, N, D)
        .astype(np.float32)
    )
    return out.reshape(B, H, N, D), res


def kernel(q, k, v):
    return run(q, k, v)[0]


# revision 8
# speedup vs baseline: 1.0320x; 1.0320x over previous
"""RBF causal attention (unnormalized, no softmax denominator) on 8 Trainium2 NeuronCores.

Problem: B=2 H=16 N=2048 D=128 fp32.
  P[m,n] = exp(-s*||q_m - k_n||^2) for m >= n else 0;  O = P @ V
         = exp(2s*(q.k) - s*|k|^2) * exp(-s*|q|^2) masked causally.

Sharding: (b*h) = 32 independent slices -> 4 per core across 8 cores, no comms.

v2: all-bf16 PE datapath + host-side exact row/col statistics.
  - host precomputes ksqb = -s*|k|^2 and eq = exp(-s*|q|^2) in fp32 from the
    ORIGINAL fp32 q/k (exact, matches reference) and ships them as tiny
    [S, N] fp32 inputs; q/k/v ship as bf16 (half the DMA bytes, and bf16
    matmul runs the PE at full rate vs f32r).
  - this deletes all on-device squares (GPSIMD), reduces (DVE) and eq-exp
    (ACT) prep work and its filler scheduling; only transposes + input DMA
    remain as pipeline fillers.

Single flat software-pipelined stream over (slice, strip, bn):
  - chunked n-major DMA loads (4-block pieces) so transposes start early
  - PE-transpose Q,K 128x128 blocks (bf16) staged in PSUM, DVE-drained to
    SBUF (bf16 = 2x DVE rate); transpose groups interleave into the ACT-bound
    bn stream as PE filler
  - two m-strips of 1024 per slice; per (strip, bn):
      ST[n, m] = KT_bn.T @ QT strip   (bf16 matmul, fp32 PSUM, 512 halves)
      PT = exp(2s*ST - s*k_sq[n])     (ACT, bf16 out, per-partition bias)
      diag block: PT *= upper-tri mask (DVE/GPSIMD, bf16)
      per m-block j >= bn: ACC[m, d] += PT_block.T @ vb_bn  (bf16 matmul,
        P^T block as stationary -> output lands directly in [m, d] layout)
      PSUM allows one open accumulation group per 2KB bank: ACC is two
      1-bank tiles (4 m-blocks each); open at the bank's first write,
      close at its last diag, then DVE-drain the quad with eq scale
  - output DMA per strip half
"""

import os
import sys

import numpy as np

_TRN_REPO = "/opt/trn_rl_repo"
if os.path.isdir(_TRN_REPO) and _TRN_REPO not in sys.path:
    sys.path.insert(0, _TRN_REPO)

import concourse.bass as bass  # noqa: E402
import concourse.mybir as mybir  # noqa: E402
import concourse.tile as tile  # noqa: E402
from concourse import bacc  # noqa: E402
from concourse.bass_utils import run_bass_kernel_spmd  # noqa: E402
from concourse.masks import make_identity, make_upper_triangular  # noqa: E402

B, H, N, D = 2, 16, 2048, 128
SM_SCALE = 0.08838834764831845  # 1/sqrt(D)
NCORES = 8
SLICES = (B * H) // NCORES  # per core
NT = N // 128  # 16 row-blocks per slice

F32 = mybir.dt.float32
BF16 = mybir.dt.bfloat16

_nc_cache = None


def _build_nc():
    nc = bacc.Bacc("TRN2", target_bir_lowering=False, debug=False, num_devices=NCORES)

    q_dram = nc.dram_tensor("q", [SLICES, N, D], BF16, kind="ExternalInput").ap()
    k_dram = nc.dram_tensor("k", [SLICES, N, D], BF16, kind="ExternalInput").ap()
    v_dram = nc.dram_tensor("v", [SLICES, N, D], BF16, kind="ExternalInput").ap()
    # exact fp32 host-side stats: ksqb = -s*|k|^2, eq = exp(-s*|q|^2)
    ksqb_dram = nc.dram_tensor("ksqb", [SLICES, N], F32, kind="ExternalInput").ap()
    eq_dram = nc.dram_tensor("eq", [SLICES, N], F32, kind="ExternalInput").ap()
    o_dram = nc.dram_tensor("o", [SLICES, N, D], BF16, kind="ExternalOutput").ap()

    with tile.TileContext(nc) as tc:
        singles = tc.alloc_tile_pool(name="singles", bufs=1)
        io = tc.alloc_tile_pool(name="io", bufs=2)
        tqk = tc.alloc_tile_pool(name="tqk", bufs=2)
        smalls = tc.alloc_tile_pool(name="smalls", bufs=2)
        ptp = tc.alloc_tile_pool(name="ptp", bufs=6)
        outp = tc.alloc_tile_pool(name="outp", bufs=2)
        # 8-bank PSUM budget: stp 3 x 2 banks (QK tiles + transpose stages
        # share the ring), accp 2 x 1 bank
        stp = tc.alloc_tile_pool(name="stp", bufs=3, space="PSUM")
        accp = tc.alloc_tile_pool(name="accp", bufs=2, space="PSUM")

        ident = singles.tile([128, 128], F32)
        make_identity(nc, ident)
        identb = singles.tile([128, 128], BF16)
        nc.vector.tensor_copy(identb, ident)
        # tri[n, m] = 1.0 where m >= n else 0.0 (keep causal, [n, m] layout)
        tri_f = singles.tile([128, 128], F32)
        make_upper_triangular(nc, tri_f, val=1.0, diag=True)
        tri_bf = singles.tile([128, 128], BF16)
        nc.vector.tensor_copy(tri_bf, tri_f)

        def dma_piece(s, dram, t0, t1, dst, eng=None):
            (eng or nc.sync).dma_start(
                out=dst[:, t0:t1],
                in_=dram[s][128 * t0 : 128 * t1].rearrange(
                    "(t p) d -> p t d", p=128
                ),
            )

        def dma_stat(s, dram, dst, eng=None):
            (eng or nc.sync).dma_start(
                out=dst,
                in_=dram[s].rearrange("(t p) -> p t", p=128),
            )

        def emit_in_dma(s):
            """First-half input DMAs (4-block pieces). All on the sync (SP)
            ring: DMA issues cost ~650ns of the issuing sequencer, which the
            scalar queue (ACT) cannot afford. Cold start borrows the idle
            scalar ring."""
            kn = io.tile([128, NT, 128], BF16, name=f"kn{s}", tag="kn")
            qn = io.tile([128, NT, 128], BF16, name=f"qn{s}", tag="qn")
            vn = io.tile([128, NT, 128], BF16, name=f"vn{s}", tag="vn")
            ksqb = smalls.tile([128, NT], F32, name=f"ksqb{s}", tag="ksqb")
            eq = smalls.tile([128, NT], F32, name=f"eq{s}", tag="eq")
            stats[s] = (ksqb, eq)
            if s == 0:
                # cold: k chain (feeds the ACT bias) on the sync ring while
                # the q chain (feeds transposes) runs on the idle scalar ring
                dma_stat(s, ksqb_dram, ksqb)
                dma_stat(s, eq_dram, eq, nc.scalar)
                dma_piece(s, k_dram, 0, 4, kn)
                dma_piece(s, q_dram, 0, 4, qn, nc.scalar)
                dma_piece(s, q_dram, 4, 8, qn, nc.scalar)
                dma_piece(s, k_dram, 4, 8, kn)
                dma_piece(s, v_dram, 0, 4, vn, nc.scalar)
                dma_piece(s, v_dram, 4, 8, vn, nc.scalar)
                return kn, qn, vn
            dma_piece(s, q_dram, 0, 4, qn)
            dma_piece(s, q_dram, 4, 8, qn)
            dma_piece(s, k_dram, 0, 4, kn)
            dma_stat(s, ksqb_dram, ksqb)
            dma_stat(s, eq_dram, eq)
            dma_piece(s, v_dram, 0, 4, vn)
            dma_piece(s, k_dram, 4, 8, kn)
            dma_piece(s, v_dram, 4, 8, vn)
            return kn, qn, vn

        def emit_in_dma2(s):
            """Second-half input DMAs."""
            kn, qn, vn = in_tiles[s]
            dma_piece(s, k_dram, 8, 12, kn)
            dma_piece(s, k_dram, 12, 16, kn)
            dma_piece(s, q_dram, 8, 12, qn)
            dma_piece(s, q_dram, 12, 16, qn)
            sc = nc.scalar if s == 0 else nc.sync
            dma_piece(s, v_dram, 8, 12, vn, sc)
            dma_piece(s, v_dram, 12, 16, vn, sc)

        def transpose_group(s, which, g):
            """PE-transpose blocks 4g..4g+3 of kn/qn into kt/qt[:, 512g:...]
            via a PSUM stage (shared stp ring), drained by DVE (bf16 2x)."""
            src = sl[s]["io"][0] if which == "k" else sl[s]["io"][1]
            dst = sl[s]["kt"] if which == "k" else sl[s]["qt"]
            stg = stp.tile([128, 512], BF16, name=f"tsg{s}_{which}_{g}", tag="st")
            for j in range(4):
                nc.tensor.transpose(
                    stg[:, 128 * j : 128 * (j + 1)], src[:, 4 * g + j, :], identb
                )
            nc.vector.tensor_copy(dst[:, 512 * g : 512 * (g + 1)], stg)

        sl = {}
        stats = {}

        def alloc_slice(s):
            ksqb, eq = stats.pop(s)
            sl[s] = dict(
                io=in_tiles.pop(s),
                kt=tqk.tile([128, N], BF16, name=f"kt{s}", tag="kt"),
                qt=tqk.tile([128, N], BF16, name=f"qt{s}", tag="qt"),
                ksqb=ksqb,
                eq=eq,
                o_out=outp.tile([128, NT, 128], BF16, name=f"oo{s}", tag="oo"),
            )

        def qk_exp(s, p, bn):
            """ST = KT_bn.T @ QT strip; PT = bf16 exp(2s*ST - s*k_sq); mask."""
            kt, qt = sl[s]["kt"], sl[s]["qt"]
            off = max(0, 128 * bn - 1024 * p)
            stt = stp.tile([128, 1024], F32, name=f"st{s}_{p}_{bn}", tag="st")
            a = off
            while a < 1024:
                b = 512 if a < 512 else 1024
                nc.tensor.matmul(
                    stt[:, a:b],
                    kt[:, 128 * bn : 128 * (bn + 1)],
                    qt[:, 1024 * p + a : 1024 * p + b],
                    start=True,
                    stop=True,
                )
                a = b
            ptt = ptp.tile([128, 1024], BF16, name=f"pt{s}_{p}_{bn}", tag="pt")
            nc.scalar.activation(
                ptt[:, off:], stt[:, off:], mybir.ActivationFunctionType.Exp,
                bias=sl[s]["ksqb"][:, bn : bn + 1], scale=2.0 * SM_SCALE,
            )
            if bn >= 8 * p:
                eng = nc.vector if p == 0 else nc.gpsimd
                eng.tensor_mul(
                    ptt[:, off : off + 128], ptt[:, off : off + 128], tri_bf
                )
            return ptt

        # flat (s, p, bn) pair stream
        pairs = [
            (s, p, bn) for s in range(SLICES) for p in (0, 1)
            for bn in range(8 * p + 8)
        ]
        pidx = {t: i for i, t in enumerate(pairs)}

        fillers = {}

        def add_filler(key, fn):
            fillers.setdefault(pidx[key], []).append(fn)

        for s in range(SLICES):
            # second-half transposes run in this slice's strip0 (qt second
            # half needed at (s,1,0); kt blocks 8+ needed at (s,1,8))
            add_filler((s, 0, 2), lambda s=s: transpose_group(s, "k", 2))
            add_filler((s, 0, 3), lambda s=s: transpose_group(s, "k", 3))
            add_filler((s, 0, 4), lambda s=s: transpose_group(s, "q", 2))
            add_filler((s, 0, 5), lambda s=s: transpose_group(s, "q", 3))
            if s + 1 < SLICES:
                add_filler((s, 0, 0), lambda s=s: in_tiles.update(
                    {s + 1: emit_in_dma(s + 1)}))
                add_filler((s, 0, 1), lambda s=s: emit_in_dma2(s + 1))
                add_filler((s, 1, 0), lambda s=s: alloc_slice(s + 1))
                add_filler((s, 1, 2), lambda s=s: transpose_group(s + 1, "q", 0))
                add_filler((s, 1, 4), lambda s=s: transpose_group(s + 1, "k", 0))
                add_filler((s, 1, 6), lambda s=s: transpose_group(s + 1, "q", 1))
                add_filler((s, 1, 8), lambda s=s: transpose_group(s + 1, "k", 1))

        # cold start: slice 0 DMA, first-half transposes
        in_tiles = {}
        in_tiles[0] = emit_in_dma(0)
        emit_in_dma2(0)
        alloc_slice(0)
        transpose_group(0, "q", 0)
        transpose_group(0, "k", 0)
        transpose_group(0, "q", 1)
        transpose_group(0, "k", 1)

        def emit_out(s, p):
            nc.sync.dma_start(
                out=o_dram[s][1024 * p : 1024 * (p + 1)].rearrange(
                    "(t p2) d -> p2 t d", p2=128
                ),
                in_=sl[s]["o_out"][:, 8 * p : 8 * (p + 1)],
            )

        outq = []
        acc = {}
        pts = {0: qk_exp(*pairs[0]), 1: qk_exp(*pairs[1])}
        for i, (s, p, bn) in enumerate(pairs):
            if i + 2 < len(pairs):
                pts[i + 2] = qk_exp(*pairs[i + 2])
            ptt = pts.pop(i)
            if bn == 0:
                acc[0] = accp.tile([128, 4, 128], F32, name=f"acA{s}_{p}", tag="acc")
                acc[1] = accp.tile([128, 4, 128], F32, name=f"acB{s}_{p}", tag="acc")
            j0 = max(0, bn - 8 * p)
            js = list(range(j0, 8))
            if bn >= max(8 * p, 1) and len(js) > 1:
                # diag block last so its mask is off the PE critical path
                js = js[1:] + js[:1]
            for j in js:
                bm = 8 * p + j
                bank_last = (j % 4 == 3) and bn == bm
                nc.tensor.matmul(
                    acc[j // 4][:, j % 4, :],
                    ptt[:, 128 * j : 128 * (j + 1)],
                    sl[s]["io"][2][:, bn, :],
                    start=(bn == 0 and j % 4 == 0),
                    stop=bank_last,
                )
                if bank_last:
                    for jj in range(j - 3, j + 1):
                        nc.vector.tensor_scalar_mul(
                            sl[s]["o_out"][:, 8 * p + jj, :],
                            acc[j // 4][:, jj % 4, :],
                            sl[s]["eq"][:, 8 * p + jj : 8 * p + jj + 1],
                        )
            for fn in fillers.get(i, ()):
                fn()
            if outq and outq[0][0] + 2 <= i:
                _, os_, op_ = outq.pop(0)
                emit_out(os_, op_)
            if bn == 8 * p + 7:  # strip end -> queue output DMA for this half
                outq.append((i, s, p))

        for _, os_, op_ in outq:
            # tail strips: split per bank-quad so the first half's DMA
            # overlaps the final PVs/drains instead of waiting for them
            for quad in (2 * op_, 2 * op_ + 1):
                nc.sync.dma_start(
                    out=o_dram[os_][512 * quad : 512 * (quad + 1)].rearrange(
                        "(t p2) d -> p2 t d", p2=128
                    ),
                    in_=sl[os_]["o_out"][:, 4 * quad : 4 * (quad + 1)],
                )

        for pool in (accp, stp, outp, ptp, smalls, tqk, io, singles):
            pool.release()

    nc.compile()
    return nc


def _get_nc():
    global _nc_cache
    if _nc_cache is None:
        _nc_cache = _build_nc()
    return _nc_cache


def run(q, k, v, trace=False):
    q = np.ascontiguousarray(np.asarray(q, dtype=np.float32))
    k = np.ascontiguousarray(np.asarray(k, dtype=np.float32))
    v = np.ascontiguousarray(np.asarray(v, dtype=np.float32))
    import ml_dtypes

    qf = q.reshape(B * H, N, D)
    kf = k.reshape(B * H, N, D)
    # exact fp32 statistics from the original fp32 data
    ksqb = (-SM_SCALE * np.einsum("snd,snd->sn", kf, kf)).astype(np.float32)
    eq = np.exp(-SM_SCALE * np.einsum("snd,snd->sn", qf, qf)).astype(np.float32)
    qb = qf.astype(ml_dtypes.bfloat16)
    kb = kf.astype(ml_dtypes.bfloat16)
    vb = v.reshape(B * H, N, D).astype(ml_dtypes.bfloat16)
    nc = _get_nc()
    in_maps = [
        {
            "q": np.ascontiguousarray(qb[SLICES * i : SLICES * (i + 1)]),
            "k": np.ascontiguousarray(kb[SLICES * i : SLICES * (i + 1)]),
            "v": np.ascontiguousarray(vb[SLICES * i : SLICES * (i + 1)]),
            "ksqb": np.ascontiguousarray(ksqb[SLICES * i : SLICES * (i + 1)]),
            "eq": np.ascontiguousarray(eq[SLICES * i : SLICES * (i + 1)]),
        }
        for i in range(NCORES)
    ]
    res = run_bass_kernel_spmd(nc, in_maps, core_ids=list(range(NCORES)), trace=trace)
    out = np.concatenate(
        [np.asarray(res.results[i]["o"]).astype(np.float32) for i in range(NCORES)],
        axis=0,
    )
    return out.reshape(B, H, N, D), res


def kernel(q, k, v):
    return run(q, k, v)[0]
